# revision 11
# baseline (speedup 1.0000x reference)
"""CombPool2d Trainium2 kernel.

out = (w_avg**2) * avg_pool2x2(x) + (w_max**2) * max_pool2x2(x)
x: (16, 192, 224, 224) f32, w_avg/w_max: (1, 192, 1, 1) f32.

Sharding: data-parallel over batch — 2 batches per NeuronCore on 8 cores.

Layout trick: flatten (C, H) so that each output row (one (c, j) pair,
112 output pixels) is produced from 448 contiguous input floats (input
rows 2j and 2j+1 of channel c are adjacent in DRAM).  Per batch there
are 192*112 = 21504 such row-pairs; tile them as 12 tiles of
(128 partitions x 14 row-pairs).  Each DMA is then a fully contiguous
3.2 MB HBM read per tile, and compute is pure elementwise work:

  rowsum = even_row + odd_row          (GPSIMD, contiguous)
  rowmax = max(even_row, odd_row)      (DVE,    contiguous)
  colsum = rowsum[0::2] + rowsum[1::2] (DVE,    stride-2)
  colmax = max(rowmax[0::2], rowmax[1::2])  (DVE)
  m'     = colmax * wmax2[c]           (ACT, per-partition scale)
  out    = colsum * (wavg2[c]/4) + m'  (DVE scalar_tensor_tensor)

Channel coefficients: within a tile, partition p covers one channel
c = 16*t + p//8 (112 row-pairs per channel, 14 per partition, aligned),
so coefficients are per-partition scalars, precomputed on host (192
floats of work) and DMA'd once.
"""

import json

import numpy as np

import concourse.bass as bass
import concourse.mybir as mybir
from concourse.tile import TileContext
from concourse.bass_utils import run_bass_kernel_spmd


def _split_multi_waits(bir: dict) -> dict:
    """The walrus build in this container rejects instructions carrying more
    than one semaphore wait ("Too many sync wait commands").  Engines execute
    their instruction stream in order, so hoisting all-but-one wait onto
    standalone EventSemaphore instructions inserted immediately before the
    instruction is semantically identical."""
    ctr = 0
    for fn in bir["functions"]:
        for blk in fn["blocks"]:
            out = []
            for ins in blk["instructions"]:
                si = ins.get("sync_info")
                waits = si.get("on_wait", []) if si else []
                if len(waits) > 1:
                    for w in waits[:-1]:
                        ctr += 1
                        out.append(
                            {
                                "debug": ins.get("debug", 0),
                                "engine": ins["engine"],
                                "ins": [],
                                "outs": [],
                                "name": f"{ins['name']}-sw{ctr}",
                                "opcode": "EventSemaphore",
                                "sync_info": {"on_update": [], "on_wait": [w]},
                            }
                        )
                    si["on_wait"] = [waits[-1]]
                out.append(ins)
            blk["instructions"] = out
    return bir


class _SplitWaitsBass(bass.Bass):
    def to_json_bytes(self) -> bytes:
        d = json.loads(super().to_json_bytes())
        _split_multi_waits(d)
        return json.dumps(d).encode()

B, C, H, W = 16, 192, 224, 224
OH, OW = H // 2, W // 2
NCORES = 8
BPC = B // NCORES              # batches per core
P = 128                        # SBUF partitions
KRP = 14                       # row-pairs per partition per tile
TPB = (C * OH) // (P * KRP)    # tiles per batch = 12
NT = BPC * TPB                 # tiles per core = 24
FIN = KRP * 2 * W              # input elems / partition / tile = 6272
FOUT = KRP * OW                # output elems / partition / tile = 1568

_nc_cache = []


def build_variant(
    krp=KRP, xbufs=3, rbufs=2, obufs=3, inplace_cm=False, out_on_act=False
):
    f32 = mybir.dt.float32
    tpb = (C * OH) // (P * krp)
    nt = BPC * tpb
    fin = krp * 2 * W
    fout = krp * OW
    assert 112 % krp == 0 and (C * OH) % (P * krp) == 0

    nc = _SplitWaitsBass()
    x_d = nc.dram_tensor("x", [nt, P, fin], f32, kind="ExternalInput")
    coef_d = nc.dram_tensor("coef", [P, 2 * tpb], f32, kind="ExternalInput")
    out_d = nc.dram_tensor("out", [nt, P, fout], f32, kind="ExternalOutput")

    with TileContext(nc) as tc:
        with (
            tc.tile_pool(name="cpool", bufs=1) as cpool,
            tc.tile_pool(name="xpool", bufs=xbufs) as xpool,
            tc.tile_pool(name="rpool", bufs=rbufs) as rpool,
            tc.tile_pool(name="opool", bufs=obufs) as opool,
        ):
            coef = cpool.tile([P, 2 * tpb], f32)
            nc.sync.dma_start(coef, coef_d[:, :])
            # Per-engine private copies so steady-state consumers only ever
            # wait on compute-engine sems.
            coefA = cpool.tile([P, tpb], f32)
            coefM = cpool.tile([P, tpb], f32)
            nc.vector.tensor_copy(coefA, coef[:, :tpb])
            nc.scalar.copy(coefM, coef[:, tpb:])
            for i in range(nt):
                tb = i % tpb
                xt = xpool.tile([P, fin], f32, tag="xt")
                nc.sync.dma_start(xt, x_d[i])
                x4 = xt.rearrange("p (s two w) -> p s two w", two=2, w=W)
                ev = x4[:, :, 0, :]
                od = x4[:, :, 1, :]

                rs = rpool.tile([P, krp * W], f32, tag="rs")
                rm = rpool.tile([P, krp * W], f32, tag="rm")
                nc.gpsimd.tensor_add(rs.rearrange("p (s w) -> p s w", w=W), ev, od)
                nc.vector.tensor_max(rm.rearrange("p (s w) -> p s w", w=W), ev, od)

                rs4 = rs.rearrange("p (s w two) -> p s w two", two=2, w=OW)
                rm4 = rm.rearrange("p (s w two) -> p s w two", two=2, w=OW)
                cs = rpool.tile([P, fout], f32, tag="cs")
                cm = rpool.tile([P, fout], f32, tag="cm")
                nc.vector.tensor_add(
                    cs.rearrange("p (s w) -> p s w", w=OW),
                    rs4[:, :, :, 0],
                    rs4[:, :, :, 1],
                )
                nc.vector.tensor_max(
                    cm.rearrange("p (s w) -> p s w", w=OW),
                    rm4[:, :, :, 0],
                    rm4[:, :, :, 1],
                )

                if inplace_cm:
                    cmx = cm
                    nc.scalar.mul(cmx, cm, coefM[:, tb : tb + 1])
                else:
                    cmx = rpool.tile([P, fout], f32, tag="cmx")
                    nc.scalar.mul(cmx, cm, coefM[:, tb : tb + 1])

                ot = opool.tile([P, fout], f32, tag="ot")
                nc.vector.scalar_tensor_tensor(
                    ot,
                    cs,
                    coefA[:, tb : tb + 1],
                    cmx,
                    op0=mybir.AluOpType.mult,
                    op1=mybir.AluOpType.add,
                )
                out_eng = nc.scalar if out_on_act else nc.sync
                out_eng.dma_start(out_d[i], ot)
    nc._variant = dict(krp=krp, tpb=tpb, nt=nt, fin=fin, fout=fout)
    return nc


# current best configuration used by kernel()
BEST = dict(krp=8, xbufs=5, rbufs=3, obufs=4, inplace_cm=True, out_on_act=True)


def get_nc():
    if not _nc_cache:
        _nc_cache.append(build_variant(**BEST))
    return _nc_cache[0]


def make_coef(w_avg, w_max, krp, tpb):
    ca = (np.asarray(w_avg).reshape(C).astype(np.float64) ** 2) / 4.0
    cm = np.asarray(w_max).reshape(C).astype(np.float64) ** 2
    # partition p of tile tb covers channel (tb*P*krp + p*krp) // OH
    chan = (
        np.arange(tpb)[None, :] * P * krp + np.arange(P)[:, None] * krp
    ) // OH  # (P, tpb)
    return np.concatenate([ca[chan], cm[chan]], axis=1).astype(np.float32)


def make_in_maps(x, w_avg, w_max, v):
    coef = make_coef(w_avg, w_max, v["krp"], v["tpb"])
    x = np.asarray(x)
    in_maps = []
    for c in range(NCORES):
        xc = np.ascontiguousarray(x[c * BPC : (c + 1) * BPC]).reshape(
            v["nt"], P, v["fin"]
        )
        in_maps.append({"x": xc, "coef": coef})
    return in_maps


def kernel(x, w_avg, w_max):
    nc = get_nc()
    in_maps = make_in_maps(x, w_avg, w_max, nc._variant)
    try:
        res = run_bass_kernel_spmd(nc, in_maps, core_ids=list(range(NCORES)))
    except Exception:
        # A previously-crashed run can leave the device wedged; one retry
        # after it resets is usually enough.
        import time

        time.sleep(5)
        res = run_bass_kernel_spmd(nc, in_maps, core_ids=list(range(NCORES)))
    outs = [r["out"].reshape(BPC, C, OH, OW) for r in res.results]
    return np.concatenate(outs, axis=0)
